# revision 20
# baseline (speedup 1.0000x reference)
"""DFMConv2d Trainium2 kernel (v2).

Reference computation (per sample b):
  pooled = mean_{h,w} x[b]                          [C=256]
  h      = relu(pooled @ w1.T + b1)                 [128]
  mix    = softmax((h @ w2.T + b2).reshape(256, 8)) [256, 8]
  y      = conv3x3_SAME(x[b], base_filters)         [8, 64, 64]
  out[b] = einsum('on,nhw->ohw', mix, y)            [256, 64, 64]

Strategy (8 NeuronCores, data-parallel over batch, 8 samples/core), heavy
path in bf16 (f32 PSUM accumulation):

  conv:  y_tap[(t,n), hw] = sum_c filt[t,n,c] * x[c, hw] — all 9 taps in
         the stationary M dim (M=96, rows 32*dx+8*dy+n), x streams through
         the PE twice; 16 matmuls/sample into row-padded ypad[96, 4227].
  shift: z[(t,n), hw] = y_tap shifted by (dy-1, dx-1) — one contiguous
         SBUF->SBUF DMA per tap on the SP (sync) ring + 2 gpsimd memset
         column fixups for the dx wraparound cells.
  pool:  cc0 via 8 accumulating identity matmuls on PE (+0.6us DVE reduce),
         cc1 via one gpsimd halving add + DVE reduce. Pooling depends only
         on the x load, so it runs ~2 samples ahead of the conv; the group-1
         attention MLP is ready by sample 4 and the per-sample mixes pipeline
         with no mix-only tail.
  mix:   out[o, hw] = mixT.T @ z with K=96; mixT built by replicating the
         softmax 12x along the free axis (stride-0 DVE read) + PE transpose.
  Engine split: x loads on GPSIMD/SWDGE ring, out stores on the ACT HWDGE
  ring, z shifts on the SP ring. PSUM drains alternate ACT(3):DVE(2).
  PSUM: conv 2x[128,1024] + mix 2x[128,512] + pool [128,512] +
  transpose scratch [128,512] = 8 banks.
"""
import sys

sys.path.insert(0, "/opt/trn_rl_repo")

import numpy as np
import ml_dtypes

import concourse.bass as bass
import concourse.bacc as bacc
import concourse.tile as tile
import concourse.mybir as mybir
from concourse.bass_utils import run_bass_kernel_spmd
from contextlib import ExitStack

F32 = mybir.dt.float32
BF16 = mybir.dt.bfloat16
AFT = mybir.ActivationFunctionType
AXX = mybir.AxisListType.X
ALU = mybir.AluOpType

N_CORES = 8
BPC = 8            # samples per core
G = 4              # MLP batch group size
C = 256
CO = 256
H = W = 64
HW = H * W
NB = 8             # n_base
HID = 128
CCH = 2            # channel chunks of 128
M96 = 96           # taps*bases rows: 32*dx + 8*dy + n (rows 24:32, 56:64, 88:96 zero)
YP_LEN = 1 + 66 * 64 + 2   # lead zero + 66 rows + tail slack
TAP_ROW = {(dy, dx): 32 * dx + 8 * dy for dy in range(3) for dx in range(3)}

# schedule: which samples get pooled / mixed while conv(j) runs
POOLS_AT = {0: [0, 1], 1: [2, 3], 2: [4], 3: [5], 4: [6, 7]}
MIXES_AT = {2: [0], 3: [1], 4: [2], 5: [3, 4], 6: [5], 7: [6, 7]}
MLP_AT = {1: 0, 4: 1}

_BUILT = None


def _build():
    nc = bacc.Bacc("TRN2", target_bir_lowering=False)

    d_x = nc.dram_tensor("x", [BPC, C, HW], BF16, kind="ExternalInput")
    d_w1t = nc.dram_tensor("w1t", [C, HID], F32, kind="ExternalInput")
    d_b1 = nc.dram_tensor("b1", [HID, 1], F32, kind="ExternalInput")
    d_w2p = nc.dram_tensor("w2p", [HID, NB, CO], BF16, kind="ExternalInput")
    d_b2r = nc.dram_tensor("b2r", [128, 2, G, NB], F32, kind="ExternalInput")
    d_ft = nc.dram_tensor("ft", [128, CCH, M96], BF16, kind="ExternalInput")
    d_id = nc.dram_tensor("ident", [128, 128], BF16, kind="ExternalInput")
    d_z0 = nc.dram_tensor("zeros", [128, 66], BF16, kind="ExternalInput")
    d_zz = nc.dram_tensor("zrow", [NB, HW], BF16, kind="ExternalInput")
    d_out = nc.dram_tensor("out", [BPC, 2, 128, HW], BF16, kind="ExternalOutput")

    with tile.TileContext(nc) as tc, ExitStack() as ctx:
        prm = ctx.enter_context(tc.tile_pool(name="prm", bufs=1))
        xp = ctx.enter_context(tc.tile_pool(name="xp", bufs=6))
        ypp = ctx.enter_context(tc.tile_pool(name="ypp", bufs=2))
        mt = ctx.enter_context(tc.tile_pool(name="mt", bufs=8))
        op = ctx.enter_context(tc.tile_pool(name="op", bufs=4))
        sm = ctx.enter_context(tc.tile_pool(name="sm", bufs=2))
        ps_c = ctx.enter_context(tc.tile_pool(name="ps_c", bufs=3, space="PSUM"))
        ps_m = ctx.enter_context(tc.tile_pool(name="ps_m", bufs=3, space="PSUM"))
        ps_p = ctx.enter_context(tc.tile_pool(name="ps_p", bufs=2, space="PSUM"))
        pls = ctx.enter_context(tc.tile_pool(name="pls", bufs=2))

        # ---- params (loaded once, SP ring) ----
        w1t_sb = prm.tile([128, CCH, HID], F32, tag="w1t")
        nc.sync.dma_start(out=w1t_sb, in_=d_w1t[:, :].rearrange("(cc p) h -> p cc h", p=128))
        b1_sb = prm.tile([128, 1], F32, tag="b1")
        nc.sync.dma_start(out=b1_sb, in_=d_b1[:, :])
        w2p_sb = prm.tile([HID, NB, CO], BF16, tag="w2p")
        nc.sync.dma_start(out=w2p_sb, in_=d_w2p[:, :, :])
        b2r_sb = prm.tile([128, 2, G, NB], F32, tag="b2r")
        nc.sync.dma_start(out=b2r_sb, in_=d_b2r[:, :, :, :])
        ft_sb = prm.tile([128, CCH, M96], BF16, tag="ft")
        nc.sync.dma_start(out=ft_sb, in_=d_ft[:, :, :])
        id_sb = prm.tile([128, 128], BF16, tag="ident")
        nc.sync.dma_start(out=id_sb, in_=d_id[:, :])
        z0_sb = prm.tile([128, 66], BF16, tag="z0")
        nc.sync.dma_start(out=z0_sb, in_=d_z0[:, :])
        pooled_sb = prm.tile([128, CCH, BPC], F32, tag="pooled")
        h_sb = prm.tile([128, BPC], BF16, tag="h")
        # z buffers rotate manually so the zero pad rows (24:32, 56:64,
        # 88:96) can be written once here and never touched again
        zbufs = [prm.tile([M96, HW], BF16, name=f"ztb{i}", tag=f"ztb{i}")
                 for i in range(4)]
        for zb in zbufs:
            for r0 in (24, 56, 88):
                nc.sync.dma_start(out=zb[r0:r0 + NB, :], in_=d_zz[:, :])

        xts = {}
        zts = {}
        mixTs = {}
        ypads = {}
        pps = {}
        ots = {}
        drain_ctr = [0]

        def drain(out_ap, in_ap):
            # PSUM -> SBUF drains alternate ACT(5) : DVE(4)
            k = drain_ctr[0] % 9
            drain_ctr[0] += 1
            if k % 2 == 0:
                nc.scalar.copy(out=out_ap, in_=in_ap)
            else:
                nc.vector.tensor_copy(out_ap, in_ap)

        def block_load(j):
            xt = xp.tile([128, CCH, HW], BF16, tag="x")
            xts[j] = xt
            xv = d_x[j, :, :].rearrange("(cc p) hw -> p cc hw", p=128)
            nc.gpsimd.dma_start(out=xt[:, :, 0:HW // 2], in_=xv[:, :, 0:HW // 2])
            nc.gpsimd.dma_start(out=xt[:, :, HW // 2:HW], in_=xv[:, :, HW // 2:HW])

        def pool_chunk(j, k):
            # cc0: accumulating identity matmul chunk k of 8 on PE
            # (w1t carries the 1/HW scale)
            xt = xts[j]
            if k == 0:
                pps[j] = ps_p.tile([128, 512], F32, tag="pp", name="pp")
            nc.tensor.matmul(pps[j], id_sb, xt[:, 0, 512 * k:512 * (k + 1)],
                             start=(k == 0), stop=(k == 7))
            if k == 7:
                nc.vector.reduce_sum(pooled_sb[:, 0, j:j + 1], pps.pop(j), axis=AXX)
                # cc1: gpsimd halving add + DVE reduce
                tmp = pls.tile([128, HW // 2], BF16, tag="ptree")
                nc.gpsimd.tensor_tensor(out=tmp, in0=xt[:, 1, 0:HW // 2],
                                        in1=xt[:, 1, HW // 2:HW], op=ALU.add)
                nc.vector.reduce_sum(pooled_sb[:, 1, j:j + 1], tmp, axis=AXX)

        def conv_chunk(j, k):
            # one 512-col chunk: 2 matmuls (cc0 start, cc1 stop) + drain into ypad
            xt = xts[j]
            if k == 0:
                ypad = ypp.tile([M96, YP_LEN], BF16, tag="ypad", name="ypad")
                ypads[j] = ypad
                nc.gpsimd.tensor_copy(ypad[:, 0:65], z0_sb[0:M96, 0:65])
                nc.gpsimd.tensor_copy(ypad[:, 4161:4226], z0_sb[0:M96, 0:65])
            ypad = ypads[j]
            yps = ps_c.tile([128, 512], F32, tag="yps")
            c0 = 512 * k
            for cc in range(CCH):
                nc.tensor.matmul(yps[0:M96, :], ft_sb[:, cc, :],
                                 xt[:, cc, c0:c0 + 512],
                                 start=(cc == 0), stop=(cc == 1))
            drain(ypad[:, 65 + c0:65 + c0 + 512], yps[0:M96, :])

        def conv_finish(j):
            # per-tap shifted windows into z (contiguous SBUF->SBUF DMAs)
            ypad = ypads.pop(j)
            zt = zbufs[j % 4]
            zts[j] = zt
            for dy in range(3):
                for dx in range(3):
                    r = TAP_ROW[(dy, dx)]
                    off = dy * 64 + dx
                    nc.sync.dma_start(out=zt[r:r + NB, :],
                                      in_=ypad[r:r + NB, off:off + HW])
            # zero the dx wraparound columns: col 0 for dx=0, col 63 for dx=2
            ztv = zt.rearrange("p (h w) -> p h w", w=64)
            nc.gpsimd.memset(ztv[0:24, :, 0:1].rearrange("p h w -> p (h w)"), 0.0)
            nc.gpsimd.memset(ztv[64:88, :, 63:64].rearrange("p h w -> p (h w)"), 0.0)

        def block_mlp(g):
            j0 = G * g
            # borrow one mix-psum slot: layer-2 logits at 0:64, layer-1 at 64:68
            pm = ps_m.tile([128, 512], F32, tag="om")
            ph = pm[:, 64:64 + G]
            for cc in range(CCH):
                nc.tensor.matmul(ph, w1t_sb[:, cc, :], pooled_sb[:, cc, j0:j0 + G],
                                 start=(cc == 0), stop=(cc == 1))
            nc.scalar.activation(out=h_sb[:, j0:j0 + G], in_=ph, func=AFT.Relu,
                                 bias=b1_sb, scale=1.0)
            pl = pm[:, 0:64].rearrange("p (oc g n) -> p oc g n", oc=2, g=G)
            for oc in range(2):
                for n in range(NB):
                    nc.tensor.matmul(pl[:, oc, :, n],
                                     w2p_sb[:, n, oc * 128:(oc + 1) * 128],
                                     h_sb[:, j0:j0 + G], start=True, stop=True)
            lg = sm.tile([128, 2, G, NB], F32, tag="lg")
            nc.vector.tensor_tensor(out=lg, in0=pl, in1=b2r_sb, op=ALU.add)
            ex = sm.tile([128, 2, G, NB], F32, tag="ex")
            nc.scalar.activation(out=ex, in_=lg, func=AFT.Exp)
            sums = sm.tile([128, 2, G], F32, tag="sums")
            nc.vector.reduce_sum(sums, ex, axis=AXX)
            rec = sm.tile([128, 2, G], F32, tag="rec")
            nc.vector.reciprocal(rec, sums)
            # normalized softmax replicated 12x along free axis (one DVE op)
            mixrep = sm.tile([128, 2, G, 12, NB], BF16, tag="mixrep")
            for oc in range(2):
                nc.vector.tensor_tensor(
                    out=mixrep[:, oc],
                    in0=ex[:, oc].unsqueeze(2).to_broadcast([128, G, 12, NB]),
                    in1=rec[:, oc].unsqueeze(2).unsqueeze(3).to_broadcast(
                        [128, G, 12, NB]),
                    op=ALU.mult)
            # mixT[(t,n), oc, o] via PE transpose per (sample, oc);
            # borrows the pool psum bank (free between pool accumulations)
            pt = ps_p.tile([128, 512], F32, tag="pp")
            for jj in range(G):
                mixT = mt.tile([M96, 2, 128], BF16, tag="mixT")
                mixTs[j0 + jj] = mixT
                for oc in range(2):
                    k = 2 * jj + oc
                    ptr = pt[0:M96, 64 * k:64 * (k + 1)].bitcast(BF16)
                    nc.tensor.transpose(
                        ptr, mixrep[:, oc, jj, :, :].rearrange("p a b -> p (a b)"),
                        id_sb)
                    drain(mixT[:, oc, :], ptr)

        def mix_chunk(j, c):
            # c in 0..15: oc = c // 8, hw chunk = c % 8
            oc, k = c // 8, c % 8
            if k == 0:
                ots[(j, oc)] = op.tile([128, HW], BF16, tag="out", name="ot")
            ot = ots[(j, oc)]
            om = ps_m.tile([128, 512], F32, tag="om")
            nc.tensor.matmul(om, mixTs[j][:, oc, :], zts[j][:, 512 * k:512 * (k + 1)],
                             start=True, stop=True)
            drain(ot[:, 512 * k:512 * (k + 1)], om)
            if k == 7:
                nc.scalar.dma_start(out=d_out[j, oc, :, :], in_=ots.pop((j, oc)))
                if oc == 1:
                    zts.pop(j)
                    mixTs.pop(j)

        for j in range(4):
            block_load(j)
        for j in range(BPC):
            if j + 4 < BPC:
                block_load(j + 4)
            pool_list = POOLS_AT.get(j, [])
            mix_list = MIXES_AT.get(j, [])
            # chunk-level interleave: conv chunk + pool chunk + 2 mix chunks
            for k in range(8):
                conv_chunk(j, k)
                for p in pool_list:
                    pool_chunk(p, k)
                if mix_list:
                    mix_chunk(mix_list[0], 2 * k)
                    mix_chunk(mix_list[0], 2 * k + 1)
            conv_finish(j)
            if j in MLP_AT:
                block_mlp(MLP_AT[j])
            # emit remaining mixes (second one in double-mix iters)
            for m in mix_list[1:]:
                for c in range(16):
                    mix_chunk(m, c)

    nc.compile()
    return nc


def _prep_inputs(x, w1, b1, w2, b2, base_filters):
    """Host-side input layout prep. Returns per-core in_maps."""
    B = x.shape[0]
    xs = np.ascontiguousarray(x.reshape(B, C, HW)).astype(ml_dtypes.bfloat16)
    w1t = np.ascontiguousarray(w1.T).astype(np.float32) / float(HW)
    b1c = np.ascontiguousarray(b1.reshape(HID, 1)).astype(np.float32)
    w2p = np.ascontiguousarray(
        w2.reshape(CO, NB, HID).transpose(2, 1, 0)).astype(ml_dtypes.bfloat16)
    # b2r[o_part, oc, smp, n] = b2[(oc*128 + o_part)*8 + n]
    b2r = np.broadcast_to(
        b2.reshape(2, 128, NB).transpose(1, 0, 2)[:, :, None, :],
        (128, 2, G, NB))
    b2r = np.ascontiguousarray(b2r).astype(np.float32)
    filt = base_filters.reshape(NB, CCH, 128, 3, 3)  # [n, cc, cp, dy, dx]
    # ft[c_part, cc, 32*dx + 8*dy + n] = filt[n, cc, c_part, dy, dx]
    ft = np.zeros((128, CCH, M96), dtype=np.float32)
    for dy in range(3):
        for dx in range(3):
            r = TAP_ROW[(dy, dx)]
            ft[:, :, r:r + NB] = filt[:, :, :, dy, dx].transpose(2, 1, 0)
    ft = ft.astype(ml_dtypes.bfloat16)
    ident = np.eye(128, dtype=np.float32).astype(ml_dtypes.bfloat16)
    zeros = np.zeros((128, 66), dtype=ml_dtypes.bfloat16)
    zrow = np.zeros((NB, HW), dtype=ml_dtypes.bfloat16)

    in_maps = []
    for core in range(N_CORES):
        in_maps.append({
            "x": np.ascontiguousarray(xs[core * BPC:(core + 1) * BPC]),
            "w1t": w1t, "b1": b1c, "w2p": w2p, "b2r": b2r,
            "ft": ft, "ident": ident, "zeros": zeros, "zrow": zrow,
        })
    return in_maps


def kernel(x, w1, b1, w2, b2, base_filters):
    global _BUILT
    if _BUILT is None:
        _BUILT = _build()
    nc = _BUILT
    in_maps = _prep_inputs(np.asarray(x, dtype=np.float32),
                           np.asarray(w1, dtype=np.float32),
                           np.asarray(b1, dtype=np.float32),
                           np.asarray(w2, dtype=np.float32),
                           np.asarray(b2, dtype=np.float32),
                           np.asarray(base_filters, dtype=np.float32))
    res = run_bass_kernel_spmd(nc, in_maps, core_ids=list(range(N_CORES)))
    outs = []
    for core in range(N_CORES):
        o = np.asarray(res.results[core]["out"])    # [BPC, 2, 128, HW] bf16
        outs.append(o.reshape(BPC, CO, H, W).astype(np.float32))
    return np.concatenate(outs, axis=0)


# revision 21
# speedup vs baseline: 1.0755x; 1.0755x over previous
"""DFMConv2d Trainium2 kernel (v2).

Reference computation (per sample b):
  pooled = mean_{h,w} x[b]                          [C=256]
  h      = relu(pooled @ w1.T + b1)                 [128]
  mix    = softmax((h @ w2.T + b2).reshape(256, 8)) [256, 8]
  y      = conv3x3_SAME(x[b], base_filters)         [8, 64, 64]
  out[b] = einsum('on,nhw->ohw', mix, y)            [256, 64, 64]

Strategy (8 NeuronCores, data-parallel over batch, 8 samples/core), heavy
path in bf16 (f32 PSUM accumulation):

  conv:  y_tap[(t,n), hw] = sum_c filt[t,n,c] * x[c, hw] — all 9 taps in
         the stationary M dim (M=96, rows 32*dx+8*dy+n), x streams through
         the PE twice; 16 matmuls/sample into row-padded ypad[96, 4227].
  shift: z[(t,n), hw] = y_tap shifted by (dy-1, dx-1) — one contiguous
         SBUF->SBUF DMA per tap on the SP (sync) ring + 2 gpsimd memset
         column fixups for the dx wraparound cells.
  pool:  cc0 via 8 accumulating identity matmuls on PE (+0.6us DVE reduce),
         cc1 via one gpsimd halving add + DVE reduce. Pooling depends only
         on the x load, so it runs ~2 samples ahead of the conv; the group-1
         attention MLP is ready by sample 4 and the per-sample mixes pipeline
         with no mix-only tail.
  mix:   out[o, hw] = mixT.T @ z with K=96; mixT built by replicating the
         softmax 12x along the free axis (stride-0 DVE read) + PE transpose.
  Engine split: x loads on GPSIMD/SWDGE ring, out stores on the ACT HWDGE
  ring, z shifts on the SP ring. PSUM drains alternate ACT(3):DVE(2).
  PSUM: conv 2x[128,1024] + mix 2x[128,512] + pool [128,512] +
  transpose scratch [128,512] = 8 banks.
"""
import sys

sys.path.insert(0, "/opt/trn_rl_repo")

import numpy as np
import ml_dtypes

import concourse.bass as bass
import concourse.bacc as bacc
import concourse.tile as tile
import concourse.mybir as mybir
from concourse.bass_utils import run_bass_kernel_spmd
from contextlib import ExitStack

F32 = mybir.dt.float32
BF16 = mybir.dt.bfloat16
AFT = mybir.ActivationFunctionType
AXX = mybir.AxisListType.X
ALU = mybir.AluOpType

N_CORES = 8
BPC = 8            # samples per core
G = 4              # MLP batch group size
C = 256
CO = 256
H = W = 64
HW = H * W
NB = 8             # n_base
HID = 128
CCH = 2            # channel chunks of 128
M96 = 96           # taps*bases rows: 32*dx + 8*dy + n (rows 24:32, 56:64, 88:96 zero)
YP_LEN = 1 + 66 * 64 + 2   # lead zero + 66 rows + tail slack
TAP_ROW = {(dy, dx): 32 * dx + 8 * dy for dy in range(3) for dx in range(3)}

# schedule: which samples get pooled / mixed while conv(j) runs
POOLS_AT = {0: [0, 1], 1: [2, 3], 2: [4], 3: [5], 4: [6, 7]}
MIXES_AT = {2: [0], 3: [1], 4: [2], 5: [3, 4], 6: [5], 7: [6, 7]}
MLP_AT = {1: 0, 4: 1}

_BUILT = None


def _build():
    nc = bacc.Bacc("TRN2", target_bir_lowering=False)

    d_x = nc.dram_tensor("x", [BPC, C, HW], BF16, kind="ExternalInput")
    d_w1t = nc.dram_tensor("w1t", [C, HID], F32, kind="ExternalInput")
    d_b1 = nc.dram_tensor("b1", [HID, 1], F32, kind="ExternalInput")
    d_w2p = nc.dram_tensor("w2p", [HID, NB, CO], BF16, kind="ExternalInput")
    d_b2r = nc.dram_tensor("b2r", [128, 2, G, NB], F32, kind="ExternalInput")
    d_ft = nc.dram_tensor("ft", [128, CCH, M96], BF16, kind="ExternalInput")
    d_id = nc.dram_tensor("ident", [128, 128], BF16, kind="ExternalInput")
    d_z0 = nc.dram_tensor("zeros", [128, 66], BF16, kind="ExternalInput")
    d_out = nc.dram_tensor("out", [BPC, 2, 128, HW], BF16, kind="ExternalOutput")

    with tile.TileContext(nc) as tc, ExitStack() as ctx:
        prm = ctx.enter_context(tc.tile_pool(name="prm", bufs=1))
        xp = ctx.enter_context(tc.tile_pool(name="xp", bufs=6))
        ypp = ctx.enter_context(tc.tile_pool(name="ypp", bufs=2))
        zp = ctx.enter_context(tc.tile_pool(name="zp", bufs=4))
        mt = ctx.enter_context(tc.tile_pool(name="mt", bufs=8))
        op = ctx.enter_context(tc.tile_pool(name="op", bufs=4))
        sm = ctx.enter_context(tc.tile_pool(name="sm", bufs=2))
        ps_c = ctx.enter_context(tc.tile_pool(name="ps_c", bufs=3, space="PSUM"))
        ps_m = ctx.enter_context(tc.tile_pool(name="ps_m", bufs=3, space="PSUM"))
        ps_p = ctx.enter_context(tc.tile_pool(name="ps_p", bufs=2, space="PSUM"))
        pls = ctx.enter_context(tc.tile_pool(name="pls", bufs=2))

        # ---- params (loaded once, SP ring) ----
        w1t_sb = prm.tile([128, CCH, HID], F32, tag="w1t")
        nc.sync.dma_start(out=w1t_sb, in_=d_w1t[:, :].rearrange("(cc p) h -> p cc h", p=128))
        b1_sb = prm.tile([128, 1], F32, tag="b1")
        nc.sync.dma_start(out=b1_sb, in_=d_b1[:, :])
        w2p_sb = prm.tile([HID, NB, CO], BF16, tag="w2p")
        nc.sync.dma_start(out=w2p_sb, in_=d_w2p[:, :, :])
        b2r_sb = prm.tile([128, 2, G, NB], F32, tag="b2r")
        nc.sync.dma_start(out=b2r_sb, in_=d_b2r[:, :, :, :])
        ft_sb = prm.tile([128, CCH, M96], BF16, tag="ft")
        nc.sync.dma_start(out=ft_sb, in_=d_ft[:, :, :])
        id_sb = prm.tile([128, 128], BF16, tag="ident")
        nc.sync.dma_start(out=id_sb, in_=d_id[:, :])
        z0_sb = prm.tile([128, 66], BF16, tag="z0")
        nc.sync.dma_start(out=z0_sb, in_=d_z0[:, :])
        pooled_sb = prm.tile([128, CCH, BPC], F32, tag="pooled")
        h_sb = prm.tile([128, BPC], BF16, tag="h")

        xts = {}
        zts = {}
        mixTs = {}
        ypads = {}
        pps = {}
        ots = {}
        drain_ctr = [0]

        def drain(out_ap, in_ap):
            # PSUM -> SBUF drains alternate ACT(5) : DVE(4)
            k = drain_ctr[0] % 9
            drain_ctr[0] += 1
            if k % 2 == 0:
                nc.scalar.copy(out=out_ap, in_=in_ap)
            else:
                nc.vector.tensor_copy(out_ap, in_ap)

        def block_load(j):
            xt = xp.tile([128, CCH, HW], BF16, tag="x")
            xts[j] = xt
            xv = d_x[j, :, :].rearrange("(cc p) hw -> p cc hw", p=128)
            nc.gpsimd.dma_start(out=xt[:, :, 0:HW // 2], in_=xv[:, :, 0:HW // 2])
            nc.gpsimd.dma_start(out=xt[:, :, HW // 2:HW], in_=xv[:, :, HW // 2:HW])

        def pool_chunk(j, k):
            # cc0: accumulating identity matmul chunk k of 8 on PE
            # (w1t carries the 1/HW scale)
            xt = xts[j]
            if k == 0:
                pps[j] = ps_p.tile([128, 512], F32, tag="pp", name="pp")
            nc.tensor.matmul(pps[j], id_sb, xt[:, 0, 512 * k:512 * (k + 1)],
                             start=(k == 0), stop=(k == 7))
            if k == 7:
                nc.vector.reduce_sum(pooled_sb[:, 0, j:j + 1], pps.pop(j), axis=AXX)
                # cc1: gpsimd halving add + DVE reduce
                tmp = pls.tile([128, HW // 2], BF16, tag="ptree")
                nc.gpsimd.tensor_tensor(out=tmp, in0=xt[:, 1, 0:HW // 2],
                                        in1=xt[:, 1, HW // 2:HW], op=ALU.add)
                nc.vector.reduce_sum(pooled_sb[:, 1, j:j + 1], tmp, axis=AXX)

        def conv_chunk(j, k):
            # one 512-col chunk: 2 matmuls (cc0 start, cc1 stop) + drain into ypad
            xt = xts[j]
            if k == 0:
                ypad = ypp.tile([M96, YP_LEN], BF16, tag="ypad", name="ypad")
                ypads[j] = ypad
                nc.gpsimd.tensor_copy(ypad[:, 0:65], z0_sb[0:M96, 0:65])
                nc.gpsimd.tensor_copy(ypad[:, 4161:4226], z0_sb[0:M96, 0:65])
            ypad = ypads[j]
            yps = ps_c.tile([128, 512], F32, tag="yps")
            c0 = 512 * k
            for cc in range(CCH):
                nc.tensor.matmul(yps[0:M96, :], ft_sb[:, cc, :],
                                 xt[:, cc, c0:c0 + 512],
                                 start=(cc == 0), stop=(cc == 1))
            drain(ypad[:, 65 + c0:65 + c0 + 512], yps[0:M96, :])

        def conv_finish(j):
            # per-tap shifted windows into z (contiguous SBUF->SBUF DMAs)
            ypad = ypads.pop(j)
            zt = zp.tile([M96, HW], BF16, tag="z")
            zts[j] = zt
            for dy in range(3):
                for dx in range(3):
                    r = TAP_ROW[(dy, dx)]
                    off = dy * 64 + dx
                    nr = 16 if dy == 2 else NB
                    nc.sync.dma_start(out=zt[r:r + nr, :],
                                      in_=ypad[r:r + nr, off:off + HW])
            # zero the dx wraparound columns: col 0 for dx=0, col 63 for dx=2
            ztv = zt.rearrange("p (h w) -> p h w", w=64)
            nc.gpsimd.memset(ztv[0:24, :, 0:1].rearrange("p h w -> p (h w)"), 0.0)
            nc.gpsimd.memset(ztv[64:88, :, 63:64].rearrange("p h w -> p (h w)"), 0.0)

        def block_mlp(g):
            j0 = G * g
            # borrow one mix-psum slot: layer-2 logits at 0:64, layer-1 at 64:68
            pm = ps_m.tile([128, 512], F32, tag="om")
            ph = pm[:, 64:64 + G]
            for cc in range(CCH):
                nc.tensor.matmul(ph, w1t_sb[:, cc, :], pooled_sb[:, cc, j0:j0 + G],
                                 start=(cc == 0), stop=(cc == 1))
            nc.scalar.activation(out=h_sb[:, j0:j0 + G], in_=ph, func=AFT.Relu,
                                 bias=b1_sb, scale=1.0)
            pl = pm[:, 0:64].rearrange("p (oc g n) -> p oc g n", oc=2, g=G)
            for oc in range(2):
                for n in range(NB):
                    nc.tensor.matmul(pl[:, oc, :, n],
                                     w2p_sb[:, n, oc * 128:(oc + 1) * 128],
                                     h_sb[:, j0:j0 + G], start=True, stop=True)
            lg = sm.tile([128, 2, G, NB], F32, tag="lg")
            nc.vector.tensor_tensor(out=lg, in0=pl, in1=b2r_sb, op=ALU.add)
            ex = sm.tile([128, 2, G, NB], F32, tag="ex")
            nc.scalar.activation(out=ex, in_=lg, func=AFT.Exp)
            sums = sm.tile([128, 2, G], F32, tag="sums")
            nc.vector.reduce_sum(sums, ex, axis=AXX)
            rec = sm.tile([128, 2, G], F32, tag="rec")
            nc.vector.reciprocal(rec, sums)
            # normalized softmax replicated 12x along free axis (one DVE op)
            mixrep = sm.tile([128, 2, G, 12, NB], BF16, tag="mixrep")
            for oc in range(2):
                nc.vector.tensor_tensor(
                    out=mixrep[:, oc],
                    in0=ex[:, oc].unsqueeze(2).to_broadcast([128, G, 12, NB]),
                    in1=rec[:, oc].unsqueeze(2).unsqueeze(3).to_broadcast(
                        [128, G, 12, NB]),
                    op=ALU.mult)
            # mixT[(t,n), oc, o] via PE transpose per (sample, oc);
            # borrows the pool psum bank (free between pool accumulations)
            pt = ps_p.tile([128, 512], F32, tag="pp")
            for jj in range(G):
                mixT = mt.tile([M96, 2, 128], BF16, tag="mixT")
                mixTs[j0 + jj] = mixT
                for oc in range(2):
                    k = 2 * jj + oc
                    ptr = pt[0:M96, 64 * k:64 * (k + 1)].bitcast(BF16)
                    nc.tensor.transpose(
                        ptr, mixrep[:, oc, jj, :, :].rearrange("p a b -> p (a b)"),
                        id_sb)
                    drain(mixT[:, oc, :], ptr)

        def mix_chunk(j, c):
            # c in 0..15: oc = c // 8, hw chunk = c % 8
            oc, k = c // 8, c % 8
            if k == 0:
                ots[(j, oc)] = op.tile([128, HW], BF16, tag="out", name="ot")
            ot = ots[(j, oc)]
            om = ps_m.tile([128, 512], F32, tag="om")
            nc.tensor.matmul(om, mixTs[j][:, oc, :], zts[j][:, 512 * k:512 * (k + 1)],
                             start=True, stop=True)
            drain(ot[:, 512 * k:512 * (k + 1)], om)
            if k == 7:
                nc.scalar.dma_start(out=d_out[j, oc, :, :], in_=ots.pop((j, oc)))
                if oc == 1:
                    zts.pop(j)
                    mixTs.pop(j)

        for j in range(4):
            block_load(j)
        for j in range(BPC):
            if j + 4 < BPC:
                block_load(j + 4)
            pool_list = POOLS_AT.get(j, [])
            mix_list = MIXES_AT.get(j, [])
            # chunk-level interleave: conv chunk + pool chunk + 2 mix chunks
            for k in range(8):
                conv_chunk(j, k)
                for p in pool_list:
                    pool_chunk(p, k)
                if mix_list:
                    mix_chunk(mix_list[0], 2 * k)
                    mix_chunk(mix_list[0], 2 * k + 1)
            conv_finish(j)
            if j in MLP_AT:
                block_mlp(MLP_AT[j])
            # emit remaining mixes (second one in double-mix iters)
            for m in mix_list[1:]:
                for c in range(16):
                    mix_chunk(m, c)

    nc.compile()
    return nc


def _prep_inputs(x, w1, b1, w2, b2, base_filters):
    """Host-side input layout prep. Returns per-core in_maps."""
    B = x.shape[0]
    xs = np.ascontiguousarray(x.reshape(B, C, HW)).astype(ml_dtypes.bfloat16)
    w1t = np.ascontiguousarray(w1.T).astype(np.float32) / float(HW)
    b1c = np.ascontiguousarray(b1.reshape(HID, 1)).astype(np.float32)
    w2p = np.ascontiguousarray(
        w2.reshape(CO, NB, HID).transpose(2, 1, 0)).astype(ml_dtypes.bfloat16)
    # b2r[o_part, oc, smp, n] = b2[(oc*128 + o_part)*8 + n]
    b2r = np.broadcast_to(
        b2.reshape(2, 128, NB).transpose(1, 0, 2)[:, :, None, :],
        (128, 2, G, NB))
    b2r = np.ascontiguousarray(b2r).astype(np.float32)
    filt = base_filters.reshape(NB, CCH, 128, 3, 3)  # [n, cc, cp, dy, dx]
    # ft[c_part, cc, 32*dx + 8*dy + n] = filt[n, cc, c_part, dy, dx]
    ft = np.zeros((128, CCH, M96), dtype=np.float32)
    for dy in range(3):
        for dx in range(3):
            r = TAP_ROW[(dy, dx)]
            ft[:, :, r:r + NB] = filt[:, :, :, dy, dx].transpose(2, 1, 0)
    ft = ft.astype(ml_dtypes.bfloat16)
    ident = np.eye(128, dtype=np.float32).astype(ml_dtypes.bfloat16)
    zeros = np.zeros((128, 66), dtype=ml_dtypes.bfloat16)

    in_maps = []
    for core in range(N_CORES):
        in_maps.append({
            "x": np.ascontiguousarray(xs[core * BPC:(core + 1) * BPC]),
            "w1t": w1t, "b1": b1c, "w2p": w2p, "b2r": b2r,
            "ft": ft, "ident": ident, "zeros": zeros,
        })
    return in_maps


def kernel(x, w1, b1, w2, b2, base_filters):
    global _BUILT
    if _BUILT is None:
        _BUILT = _build()
    nc = _BUILT
    in_maps = _prep_inputs(np.asarray(x, dtype=np.float32),
                           np.asarray(w1, dtype=np.float32),
                           np.asarray(b1, dtype=np.float32),
                           np.asarray(w2, dtype=np.float32),
                           np.asarray(b2, dtype=np.float32),
                           np.asarray(base_filters, dtype=np.float32))
    res = run_bass_kernel_spmd(nc, in_maps, core_ids=list(range(N_CORES)))
    outs = []
    for core in range(N_CORES):
        o = np.asarray(res.results[core]["out"])    # [BPC, 2, 128, HW] bf16
        outs.append(o.reshape(BPC, CO, H, W).astype(np.float32))
    return np.concatenate(outs, axis=0)


# revision 22
# speedup vs baseline: 1.1472x; 1.0667x over previous
"""DFMConv2d Trainium2 kernel.

Reference computation (per sample b):
  pooled = mean_{h,w} x[b]                          [C=256]
  h      = relu(pooled @ w1.T + b1)                 [128]
  mix    = softmax((h @ w2.T + b2).reshape(256, 8)) [256, 8]
  y      = conv3x3_SAME(x[b], base_filters)         [8, 64, 64]
  out[b] = einsum('on,nhw->ohw', mix, y)            [256, 64, 64]

Strategy (8 NeuronCores, data-parallel over batch, 8 samples/core), heavy
path in bf16 (f32 PSUM accumulation):

  conv:  y_tap[(t,n), hw] = sum_c filt[t,n,c] * x[c, hw] — all 9 taps in
         the stationary M dim (M=96, rows 32*dx+8*dy+n), x streams through
         the PE twice; 16 matmuls/sample into row-padded ypad[96, 4227].
  shift: z[(t,n), hw] = y_tap shifted by (dy-1, dx-1) — one contiguous
         SBUF->SBUF DMA per tap on the SP (sync) ring + 2 gpsimd memset
         column fixups for the dx wraparound cells.
  pool:  cc0 via 8 accumulating identity matmuls on PE (+DVE reduce),
         cc1 via one gpsimd halving add + DVE reduce. Pooling depends only
         on the x load, so it runs 1-2 samples ahead of the conv; the group-1
         attention MLP is ready by sample 4 and the per-sample mixes pipeline
         with only a 1-sample mix tail.
  mix:   out[o, hw] = mixT.T @ z with K=96; mixT built by replicating the
         softmax 12x along the free axis (stride-0 DVE read) + PE transpose.
  Schedule: conv / pool / mix are emitted chunk-interleaved at 512-column
  granularity (POOLS_AT / MIXES_AT / MLP_AT maps) so the PE queue always
  has runnable matmuls and the HAM clock stays at 2.4 GHz.
  Engine split: x loads on GPSIMD/SWDGE ring, out stores on the ACT HWDGE
  ring, z shifts on the SP ring. PSUM drains alternate ACT(5):DVE(4).
  PSUM: conv 3x[128,512] + mix 3x[128,512] + pool/transpose 2x[128,512]
  = 8 banks.
"""
import sys

sys.path.insert(0, "/opt/trn_rl_repo")

import numpy as np
import ml_dtypes

import concourse.bass as bass
import concourse.bacc as bacc
import concourse.tile as tile
import concourse.mybir as mybir
from concourse.bass_utils import run_bass_kernel_spmd
from contextlib import ExitStack

F32 = mybir.dt.float32
BF16 = mybir.dt.bfloat16
AFT = mybir.ActivationFunctionType
AXX = mybir.AxisListType.X
ALU = mybir.AluOpType

N_CORES = 8
BPC = 8            # samples per core
G = 4              # MLP batch group size
C = 256
CO = 256
H = W = 64
HW = H * W
NB = 8             # n_base
HID = 128
CCH = 2            # channel chunks of 128
M96 = 96           # taps*bases rows: 32*dx + 8*dy + n (rows 24:32, 56:64, 88:96 zero)
YP_LEN = 1 + 66 * 64 + 2   # lead zero + 66 rows + tail slack
TAP_ROW = {(dy, dx): 32 * dx + 8 * dy for dy in range(3) for dx in range(3)}

# schedule: which samples get pooled / mixed while conv(j) runs
POOLS_AT = {0: [0, 1], 1: [2, 3], 2: [4], 3: [5], 4: [6, 7]}
MIXES_AT = {2: [0], 3: [1], 4: [2], 5: [3, 4], 6: [5], 7: [6, 7]}
MLP_AT = {1: 0, 4: 1}

_BUILT = None


def _build():
    nc = bacc.Bacc("TRN2", target_bir_lowering=False)

    d_x = nc.dram_tensor("x", [BPC, C, HW], BF16, kind="ExternalInput")
    d_w1t = nc.dram_tensor("w1t", [C, HID], F32, kind="ExternalInput")
    d_b1 = nc.dram_tensor("b1", [HID, 1], F32, kind="ExternalInput")
    d_w2p = nc.dram_tensor("w2p", [HID, NB, CO], BF16, kind="ExternalInput")
    d_b2r = nc.dram_tensor("b2r", [128, 2, G, NB], F32, kind="ExternalInput")
    d_ft = nc.dram_tensor("ft", [128, CCH, M96], BF16, kind="ExternalInput")
    d_id = nc.dram_tensor("ident", [128, 128], BF16, kind="ExternalInput")
    d_z0 = nc.dram_tensor("zeros", [128, 66], BF16, kind="ExternalInput")
    d_out = nc.dram_tensor("out", [BPC, 2, 128, HW], BF16, kind="ExternalOutput")

    with tile.TileContext(nc) as tc, ExitStack() as ctx:
        prm = ctx.enter_context(tc.tile_pool(name="prm", bufs=1))
        xp = ctx.enter_context(tc.tile_pool(name="xp", bufs=6))
        ypp = ctx.enter_context(tc.tile_pool(name="ypp", bufs=2))
        zp = ctx.enter_context(tc.tile_pool(name="zp", bufs=4))
        mt = ctx.enter_context(tc.tile_pool(name="mt", bufs=8))
        op = ctx.enter_context(tc.tile_pool(name="op", bufs=4))
        sm = ctx.enter_context(tc.tile_pool(name="sm", bufs=2))
        ps_c = ctx.enter_context(tc.tile_pool(name="ps_c", bufs=3, space="PSUM"))
        ps_m = ctx.enter_context(tc.tile_pool(name="ps_m", bufs=3, space="PSUM"))
        ps_p = ctx.enter_context(tc.tile_pool(name="ps_p", bufs=2, space="PSUM"))
        pls = ctx.enter_context(tc.tile_pool(name="pls", bufs=2))

        # ---- params (loaded once, SP ring) ----
        w1t_sb = prm.tile([128, CCH, HID], F32, tag="w1t")
        nc.sync.dma_start(out=w1t_sb, in_=d_w1t[:, :].rearrange("(cc p) h -> p cc h", p=128))
        b1_sb = prm.tile([128, 1], F32, tag="b1")
        nc.sync.dma_start(out=b1_sb, in_=d_b1[:, :])
        w2p_sb = prm.tile([HID, NB, CO], BF16, tag="w2p")
        nc.sync.dma_start(out=w2p_sb, in_=d_w2p[:, :, :])
        b2r_sb = prm.tile([128, 2, G, NB], F32, tag="b2r")
        nc.sync.dma_start(out=b2r_sb, in_=d_b2r[:, :, :, :])
        ft_sb = prm.tile([128, CCH, M96], BF16, tag="ft")
        nc.sync.dma_start(out=ft_sb, in_=d_ft[:, :, :])
        id_sb = prm.tile([128, 128], BF16, tag="ident")
        nc.sync.dma_start(out=id_sb, in_=d_id[:, :])
        z0_sb = prm.tile([128, 66], BF16, tag="z0")
        nc.sync.dma_start(out=z0_sb, in_=d_z0[:, :])
        pooled_sb = prm.tile([128, CCH, BPC], F32, tag="pooled")
        h_sb = prm.tile([128, BPC], BF16, tag="h")

        xts = {}
        zts = {}
        mixTs = {}
        ypads = {}
        pps = {}
        ots = {}
        drain_ctr = [0]

        def drain(out_ap, in_ap):
            # PSUM -> SBUF drains alternate ACT(5) : DVE(4)
            k = drain_ctr[0] % 9
            drain_ctr[0] += 1
            if k % 2 == 0:
                nc.scalar.copy(out=out_ap, in_=in_ap)
            else:
                nc.vector.tensor_copy(out_ap, in_ap)

        def block_load(j):
            xt = xp.tile([128, CCH, HW], BF16, tag="x")
            xts[j] = xt
            xv = d_x[j, :, :].rearrange("(cc p) hw -> p cc hw", p=128)
            nc.gpsimd.dma_start(out=xt[:, :, 0:HW // 2], in_=xv[:, :, 0:HW // 2])
            nc.gpsimd.dma_start(out=xt[:, :, HW // 2:HW], in_=xv[:, :, HW // 2:HW])

        def pool_chunk(j, k):
            # cc0: accumulating identity matmul chunk k of 8 on PE
            # (w1t carries the 1/HW scale)
            xt = xts[j]
            if k == 0:
                pps[j] = ps_p.tile([128, 512], F32, tag="pp", name="pp")
            nc.tensor.matmul(pps[j], id_sb, xt[:, 0, 512 * k:512 * (k + 1)],
                             start=(k == 0), stop=(k == 7))
            if k == 7:
                nc.vector.reduce_sum(pooled_sb[:, 0, j:j + 1], pps.pop(j), axis=AXX)
                # cc1: gpsimd halving add + DVE reduce
                tmp = pls.tile([128, HW // 2], BF16, tag="ptree")
                nc.gpsimd.tensor_tensor(out=tmp, in0=xt[:, 1, 0:HW // 2],
                                        in1=xt[:, 1, HW // 2:HW], op=ALU.add)
                nc.vector.reduce_sum(pooled_sb[:, 1, j:j + 1], tmp, axis=AXX)

        def conv_chunk(j, k):
            # one 512-col chunk: 2 matmuls (cc0 start, cc1 stop) + drain into ypad
            xt = xts[j]
            if k == 0:
                ypad = ypp.tile([M96, YP_LEN], BF16, tag="ypad", name="ypad")
                ypads[j] = ypad
                nc.gpsimd.tensor_copy(ypad[:, 0:65], z0_sb[0:M96, 0:65])
                nc.gpsimd.tensor_copy(ypad[:, 4161:4226], z0_sb[0:M96, 0:65])
            ypad = ypads[j]
            yps = ps_c.tile([128, 512], F32, tag="yps")
            c0 = 512 * k
            for cc in range(CCH):
                nc.tensor.matmul(yps[0:M96, :], ft_sb[:, cc, :],
                                 xt[:, cc, c0:c0 + 512],
                                 start=(cc == 0), stop=(cc == 1))
            drain(ypad[:, 65 + c0:65 + c0 + 512], yps[0:M96, :])

        def conv_finish(j):
            # per-tap shifted windows into z (contiguous SBUF->SBUF DMAs)
            ypad = ypads.pop(j)
            zt = zp.tile([M96, HW], BF16, tag="z")
            zts[j] = zt
            for dy in range(3):
                for dx in range(3):
                    r = TAP_ROW[(dy, dx)]
                    off = dy * 64 + dx
                    nr = 16 if dy == 2 else NB
                    nc.sync.dma_start(out=zt[r:r + nr, :],
                                      in_=ypad[r:r + nr, off:off + HW])
            # zero the dx wraparound columns: col 0 for dx=0, col 63 for dx=2
            ztv = zt.rearrange("p (h w) -> p h w", w=64)
            nc.gpsimd.memset(ztv[0:24, :, 0:1].rearrange("p h w -> p (h w)"), 0.0)
            nc.gpsimd.memset(ztv[64:88, :, 63:64].rearrange("p h w -> p (h w)"), 0.0)

        def block_mlp(g):
            j0 = G * g
            # borrow one mix-psum slot: layer-2 logits at 0:64, layer-1 at 64:68
            pm = ps_m.tile([128, 512], F32, tag="om")
            ph = pm[:, 64:64 + G]
            for cc in range(CCH):
                nc.tensor.matmul(ph, w1t_sb[:, cc, :], pooled_sb[:, cc, j0:j0 + G],
                                 start=(cc == 0), stop=(cc == 1))
            nc.scalar.activation(out=h_sb[:, j0:j0 + G], in_=ph, func=AFT.Relu,
                                 bias=b1_sb, scale=1.0)
            pl = pm[:, 0:64].rearrange("p (oc g n) -> p oc g n", oc=2, g=G)
            for oc in range(2):
                for n in range(NB):
                    nc.tensor.matmul(pl[:, oc, :, n],
                                     w2p_sb[:, n, oc * 128:(oc + 1) * 128],
                                     h_sb[:, j0:j0 + G], start=True, stop=True)
            lg = sm.tile([128, 2, G, NB], F32, tag="lg")
            nc.vector.tensor_tensor(out=lg, in0=pl, in1=b2r_sb, op=ALU.add)
            ex = sm.tile([128, 2, G, NB], F32, tag="ex")
            nc.scalar.activation(out=ex, in_=lg, func=AFT.Exp)
            sums = sm.tile([128, 2, G], F32, tag="sums")
            nc.vector.reduce_sum(sums, ex, axis=AXX)
            rec = sm.tile([128, 2, G], F32, tag="rec")
            nc.vector.reciprocal(rec, sums)
            # normalized softmax replicated 12x along free axis (one DVE op)
            mixrep = sm.tile([128, 2, G, 12, NB], BF16, tag="mixrep")
            for oc in range(2):
                nc.vector.tensor_tensor(
                    out=mixrep[:, oc],
                    in0=ex[:, oc].unsqueeze(2).to_broadcast([128, G, 12, NB]),
                    in1=rec[:, oc].unsqueeze(2).unsqueeze(3).to_broadcast(
                        [128, G, 12, NB]),
                    op=ALU.mult)
            # mixT[(t,n), oc, o] via PE transpose per (sample, oc);
            # borrows the pool psum bank (free between pool accumulations)
            pt = ps_p.tile([128, 512], F32, tag="pp")
            for jj in range(G):
                mixT = mt.tile([M96, 2, 128], BF16, tag="mixT")
                mixTs[j0 + jj] = mixT
                for oc in range(2):
                    k = 2 * jj + oc
                    ptr = pt[0:M96, 64 * k:64 * (k + 1)].bitcast(BF16)
                    nc.tensor.transpose(
                        ptr, mixrep[:, oc, jj, :, :].rearrange("p a b -> p (a b)"),
                        id_sb)
                    drain(mixT[:, oc, :], ptr)

        def mix_chunk(j, c):
            # c in 0..15: oc = c // 8, hw chunk = c % 8
            oc, k = c // 8, c % 8
            if k == 0:
                ots[(j, oc)] = op.tile([128, HW], BF16, tag="out", name="ot")
            ot = ots[(j, oc)]
            om = ps_m.tile([128, 512], F32, tag="om")
            nc.tensor.matmul(om, mixTs[j][:, oc, :], zts[j][:, 512 * k:512 * (k + 1)],
                             start=True, stop=True)
            drain(ot[:, 512 * k:512 * (k + 1)], om)
            if k == 7:
                nc.scalar.dma_start(out=d_out[j, oc, :, :], in_=ots.pop((j, oc)))
                if oc == 1:
                    zts.pop(j)
                    mixTs.pop(j)

        for j in range(4):
            block_load(j)
        for j in range(BPC):
            if j + 4 < BPC:
                block_load(j + 4)
            pool_list = POOLS_AT.get(j, [])
            mix_list = MIXES_AT.get(j, [])
            # chunk-level interleave: conv chunk + pool chunk + 2 mix chunks
            for k in range(8):
                conv_chunk(j, k)
                for p in pool_list:
                    pool_chunk(p, k)
                if mix_list:
                    mix_chunk(mix_list[0], 2 * k)
                    mix_chunk(mix_list[0], 2 * k + 1)
            conv_finish(j)
            if j in MLP_AT:
                block_mlp(MLP_AT[j])
            # emit remaining mixes (second one in double-mix iters)
            for m in mix_list[1:]:
                for c in range(16):
                    mix_chunk(m, c)

    nc.compile()
    return nc


def _prep_inputs(x, w1, b1, w2, b2, base_filters):
    """Host-side input layout prep. Returns per-core in_maps."""
    B = x.shape[0]
    xs = np.ascontiguousarray(x.reshape(B, C, HW)).astype(ml_dtypes.bfloat16)
    w1t = np.ascontiguousarray(w1.T).astype(np.float32) / float(HW)
    b1c = np.ascontiguousarray(b1.reshape(HID, 1)).astype(np.float32)
    w2p = np.ascontiguousarray(
        w2.reshape(CO, NB, HID).transpose(2, 1, 0)).astype(ml_dtypes.bfloat16)
    # b2r[o_part, oc, smp, n] = b2[(oc*128 + o_part)*8 + n]
    b2r = np.broadcast_to(
        b2.reshape(2, 128, NB).transpose(1, 0, 2)[:, :, None, :],
        (128, 2, G, NB))
    b2r = np.ascontiguousarray(b2r).astype(np.float32)
    filt = base_filters.reshape(NB, CCH, 128, 3, 3)  # [n, cc, cp, dy, dx]
    # ft[c_part, cc, 32*dx + 8*dy + n] = filt[n, cc, c_part, dy, dx]
    ft = np.zeros((128, CCH, M96), dtype=np.float32)
    for dy in range(3):
        for dx in range(3):
            r = TAP_ROW[(dy, dx)]
            ft[:, :, r:r + NB] = filt[:, :, :, dy, dx].transpose(2, 1, 0)
    ft = ft.astype(ml_dtypes.bfloat16)
    ident = np.eye(128, dtype=np.float32).astype(ml_dtypes.bfloat16)
    zeros = np.zeros((128, 66), dtype=ml_dtypes.bfloat16)

    in_maps = []
    for core in range(N_CORES):
        in_maps.append({
            "x": np.ascontiguousarray(xs[core * BPC:(core + 1) * BPC]),
            "w1t": w1t, "b1": b1c, "w2p": w2p, "b2r": b2r,
            "ft": ft, "ident": ident, "zeros": zeros,
        })
    return in_maps


def kernel(x, w1, b1, w2, b2, base_filters):
    global _BUILT
    if _BUILT is None:
        _BUILT = _build()
    nc = _BUILT
    in_maps = _prep_inputs(np.asarray(x, dtype=np.float32),
                           np.asarray(w1, dtype=np.float32),
                           np.asarray(b1, dtype=np.float32),
                           np.asarray(w2, dtype=np.float32),
                           np.asarray(b2, dtype=np.float32),
                           np.asarray(base_filters, dtype=np.float32))
    res = run_bass_kernel_spmd(nc, in_maps, core_ids=list(range(N_CORES)))
    outs = []
    for core in range(N_CORES):
        o = np.asarray(res.results[core]["out"])    # [BPC, 2, 128, HW] bf16
        outs.append(o.reshape(BPC, CO, H, W).astype(np.float32))
    return np.concatenate(outs, axis=0)


# revision 23
# speedup vs baseline: 1.1524x; 1.0045x over previous
"""DFMConv2d Trainium2 kernel.

Reference computation (per sample b):
  pooled = mean_{h,w} x[b]                          [C=256]
  h      = relu(pooled @ w1.T + b1)                 [128]
  mix    = softmax((h @ w2.T + b2).reshape(256, 8)) [256, 8]
  y      = conv3x3_SAME(x[b], base_filters)         [8, 64, 64]
  out[b] = einsum('on,nhw->ohw', mix, y)            [256, 64, 64]

Strategy (8 NeuronCores, data-parallel over batch, 8 samples/core), heavy
path in bf16 (f32 PSUM accumulation):

  conv:  y_tap[(t,n), hw] = sum_c filt[t,n,c] * x[c, hw] — all 9 taps in
         the stationary M dim (M=96, rows 32*dx+8*dy+n), x streams through
         the PE twice; 16 matmuls/sample into row-padded ypad[96, 4227].
  shift: z[(t,n), hw] = y_tap shifted by (dy-1, dx-1) — one contiguous
         SBUF->SBUF DMA per tap on the SP (sync) ring + 2 gpsimd memset
         column fixups for the dx wraparound cells.
  pool:  cc0 via 8 accumulating identity matmuls on PE (+DVE reduce),
         cc1 via one gpsimd halving add + DVE reduce. Pooling depends only
         on the x load, so it runs 1-2 samples ahead of the conv; the group-1
         attention MLP is ready by sample 4 and the per-sample mixes pipeline
         with only a 1-sample mix tail.
  mix:   out[o, hw] = mixT.T @ z with K=96; mixT built by replicating the
         softmax 12x along the free axis (stride-0 DVE read) + PE transpose.
  Schedule: conv / pool / mix are emitted chunk-interleaved at 512-column
  granularity (POOLS_AT / MIXES_AT / MLP_AT maps) so the PE queue always
  has runnable matmuls and the HAM clock stays at 2.4 GHz.
  Engine split: x loads on GPSIMD/SWDGE ring, out stores on the ACT HWDGE
  ring, z shifts on the SP ring. PSUM drains alternate ACT(5):DVE(4).
  PSUM: conv 3x[128,512] + mix 3x[128,512] + pool/transpose 2x[128,512]
  = 8 banks.
"""
import sys

sys.path.insert(0, "/opt/trn_rl_repo")

import numpy as np
import ml_dtypes

import concourse.bass as bass
import concourse.bacc as bacc
import concourse.tile as tile
import concourse.mybir as mybir
from concourse.bass_utils import run_bass_kernel_spmd
from contextlib import ExitStack

F32 = mybir.dt.float32
BF16 = mybir.dt.bfloat16
AFT = mybir.ActivationFunctionType
AXX = mybir.AxisListType.X
ALU = mybir.AluOpType

N_CORES = 8
BPC = 8            # samples per core
G = 4              # MLP batch group size
C = 256
CO = 256
H = W = 64
HW = H * W
NB = 8             # n_base
HID = 128
CCH = 2            # channel chunks of 128
M96 = 96           # taps*bases rows: 32*dx + 8*dy + n (rows 24:32, 56:64, 88:96 zero)
YP_LEN = 1 + 66 * 64 + 2   # lead zero + 66 rows + tail slack
TAP_ROW = {(dy, dx): 32 * dx + 8 * dy for dy in range(3) for dx in range(3)}

# schedule: which samples get pooled / mixed while conv(j) runs
POOLS_AT = {0: [0, 1], 1: [2, 3], 2: [4], 3: [5], 4: [6, 7]}
MIXES_AT = {2: [0], 3: [1], 4: [2], 5: [3, 4], 6: [5], 7: [6, 7]}
MLP_AT = {1: 0, 4: 1}

_BUILT = None


def _build():
    nc = bacc.Bacc("TRN2", target_bir_lowering=False)

    d_x = nc.dram_tensor("x", [BPC, C, HW], BF16, kind="ExternalInput")
    d_w1t = nc.dram_tensor("w1t", [C, HID], F32, kind="ExternalInput")
    d_b1 = nc.dram_tensor("b1", [HID, 1], F32, kind="ExternalInput")
    d_w2p = nc.dram_tensor("w2p", [HID, NB, CO], BF16, kind="ExternalInput")
    d_b2r = nc.dram_tensor("b2r", [128, 2, G, NB], F32, kind="ExternalInput")
    d_ft = nc.dram_tensor("ft", [128, CCH, M96], BF16, kind="ExternalInput")
    d_id = nc.dram_tensor("ident", [128, 128], BF16, kind="ExternalInput")
    d_z0 = nc.dram_tensor("zeros", [128, 66], BF16, kind="ExternalInput")
    d_out = nc.dram_tensor("out", [BPC, 2, 128, HW], BF16, kind="ExternalOutput")

    with tile.TileContext(nc) as tc, ExitStack() as ctx:
        prm = ctx.enter_context(tc.tile_pool(name="prm", bufs=1))
        xp = ctx.enter_context(tc.tile_pool(name="xp", bufs=6))
        ypp = ctx.enter_context(tc.tile_pool(name="ypp", bufs=2))
        zp = ctx.enter_context(tc.tile_pool(name="zp", bufs=4))
        mt = ctx.enter_context(tc.tile_pool(name="mt", bufs=8))
        op = ctx.enter_context(tc.tile_pool(name="op", bufs=4))
        sm = ctx.enter_context(tc.tile_pool(name="sm", bufs=2))
        ps_c = ctx.enter_context(tc.tile_pool(name="ps_c", bufs=3, space="PSUM"))
        ps_m = ctx.enter_context(tc.tile_pool(name="ps_m", bufs=3, space="PSUM"))
        ps_p = ctx.enter_context(tc.tile_pool(name="ps_p", bufs=2, space="PSUM"))
        pls = ctx.enter_context(tc.tile_pool(name="pls", bufs=2))

        # ---- params (loaded once, SP ring) ----
        w1t_sb = prm.tile([128, CCH, HID], F32, tag="w1t")
        nc.sync.dma_start(out=w1t_sb, in_=d_w1t[:, :].rearrange("(cc p) h -> p cc h", p=128))
        b1_sb = prm.tile([128, 1], F32, tag="b1")
        nc.sync.dma_start(out=b1_sb, in_=d_b1[:, :])
        w2p_sb = prm.tile([HID, NB, CO], BF16, tag="w2p")
        nc.sync.dma_start(out=w2p_sb, in_=d_w2p[:, :, :])
        b2r_sb = prm.tile([128, 2, G, NB], F32, tag="b2r")
        nc.sync.dma_start(out=b2r_sb, in_=d_b2r[:, :, :, :])
        ft_sb = prm.tile([128, CCH, M96], BF16, tag="ft")
        nc.sync.dma_start(out=ft_sb, in_=d_ft[:, :, :])
        id_sb = prm.tile([128, 128], BF16, tag="ident")
        nc.sync.dma_start(out=id_sb, in_=d_id[:, :])
        z0_sb = prm.tile([128, 66], BF16, tag="z0")
        nc.sync.dma_start(out=z0_sb, in_=d_z0[:, :])
        pooled_sb = prm.tile([128, CCH, BPC], F32, tag="pooled")
        h_sb = prm.tile([128, BPC], BF16, tag="h")

        xts = {}
        zts = {}
        mixTs = {}
        ypads = {}
        pps = {}
        ots = {}
        drain_ctr = [0]

        def drain(out_ap, in_ap):
            # PSUM -> SBUF drains alternate ACT(5) : DVE(4)
            k = drain_ctr[0] % 9
            drain_ctr[0] += 1
            if k % 2 == 0:
                nc.scalar.copy(out=out_ap, in_=in_ap)
            else:
                nc.vector.tensor_copy(out_ap, in_ap)

        def block_load(j):
            xt = xp.tile([128, CCH, HW], BF16, tag="x")
            xts[j] = xt
            xv = d_x[j, :, :].rearrange("(cc p) hw -> p cc hw", p=128)
            nc.gpsimd.dma_start(out=xt[:, :, 0:HW // 2], in_=xv[:, :, 0:HW // 2])
            nc.gpsimd.dma_start(out=xt[:, :, HW // 2:HW], in_=xv[:, :, HW // 2:HW])

        def pool_chunk(j, k):
            # cc0: accumulating identity matmul chunk k of 8 on PE
            # (w1t carries the 1/HW scale)
            xt = xts[j]
            if k == 0:
                pps[j] = ps_p.tile([128, 512], F32, tag="pp", name="pp")
            nc.tensor.matmul(pps[j], id_sb, xt[:, 0, 512 * k:512 * (k + 1)],
                             start=(k == 0), stop=(k == 7))
            if k == 7:
                nc.vector.reduce_sum(pooled_sb[:, 0, j:j + 1], pps.pop(j), axis=AXX)
                # cc1: halving add + DVE reduce. The group-last samples (3, 7)
                # gate the attention MLP, so their halving add runs on DVE
                # instead of queueing behind other work on gpsimd.
                tmp = pls.tile([128, HW // 2], BF16, tag="ptree")
                eng = nc.vector if j in (G - 1, BPC - 1) else nc.gpsimd
                eng.tensor_tensor(out=tmp, in0=xt[:, 1, 0:HW // 2],
                                  in1=xt[:, 1, HW // 2:HW], op=ALU.add)
                nc.vector.reduce_sum(pooled_sb[:, 1, j:j + 1], tmp, axis=AXX)

        def conv_chunk(j, k):
            # one 512-col chunk: 2 matmuls (cc0 start, cc1 stop) + drain into ypad
            xt = xts[j]
            if k == 0:
                ypad = ypp.tile([M96, YP_LEN], BF16, tag="ypad", name="ypad")
                ypads[j] = ypad
                nc.gpsimd.tensor_copy(ypad[:, 0:65], z0_sb[0:M96, 0:65])
                nc.gpsimd.tensor_copy(ypad[:, 4161:4226], z0_sb[0:M96, 0:65])
            ypad = ypads[j]
            yps = ps_c.tile([128, 512], F32, tag="yps")
            c0 = 512 * k
            for cc in range(CCH):
                nc.tensor.matmul(yps[0:M96, :], ft_sb[:, cc, :],
                                 xt[:, cc, c0:c0 + 512],
                                 start=(cc == 0), stop=(cc == 1))
            drain(ypad[:, 65 + c0:65 + c0 + 512], yps[0:M96, :])

        def conv_finish(j):
            # per-tap shifted windows into z (contiguous SBUF->SBUF DMAs)
            ypad = ypads.pop(j)
            zt = zp.tile([M96, HW], BF16, tag="z")
            zts[j] = zt
            for dy in range(3):
                for dx in range(3):
                    r = TAP_ROW[(dy, dx)]
                    off = dy * 64 + dx
                    nr = 16 if dy == 2 else NB
                    nc.sync.dma_start(out=zt[r:r + nr, :],
                                      in_=ypad[r:r + nr, off:off + HW])
            # zero the dx wraparound columns: col 0 for dx=0, col 63 for dx=2
            ztv = zt.rearrange("p (h w) -> p h w", w=64)
            nc.gpsimd.memset(ztv[0:24, :, 0:1].rearrange("p h w -> p (h w)"), 0.0)
            nc.gpsimd.memset(ztv[64:88, :, 63:64].rearrange("p h w -> p (h w)"), 0.0)

        def block_mlp(g):
            j0 = G * g
            # borrow one mix-psum slot: layer-2 logits at 0:64, layer-1 at 64:68
            pm = ps_m.tile([128, 512], F32, tag="om")
            ph = pm[:, 64:64 + G]
            for cc in range(CCH):
                nc.tensor.matmul(ph, w1t_sb[:, cc, :], pooled_sb[:, cc, j0:j0 + G],
                                 start=(cc == 0), stop=(cc == 1))
            nc.scalar.activation(out=h_sb[:, j0:j0 + G], in_=ph, func=AFT.Relu,
                                 bias=b1_sb, scale=1.0)
            pl = pm[:, 0:64].rearrange("p (oc g n) -> p oc g n", oc=2, g=G)
            for oc in range(2):
                for n in range(NB):
                    nc.tensor.matmul(pl[:, oc, :, n],
                                     w2p_sb[:, n, oc * 128:(oc + 1) * 128],
                                     h_sb[:, j0:j0 + G], start=True, stop=True)
            lg = sm.tile([128, 2, G, NB], F32, tag="lg")
            nc.vector.tensor_tensor(out=lg, in0=pl, in1=b2r_sb, op=ALU.add)
            ex = sm.tile([128, 2, G, NB], F32, tag="ex")
            nc.scalar.activation(out=ex, in_=lg, func=AFT.Exp)
            sums = sm.tile([128, 2, G], F32, tag="sums")
            nc.vector.reduce_sum(sums, ex, axis=AXX)
            rec = sm.tile([128, 2, G], F32, tag="rec")
            nc.vector.reciprocal(rec, sums)
            # normalized softmax replicated 12x along free axis (one DVE op)
            mixrep = sm.tile([128, 2, G, 12, NB], BF16, tag="mixrep")
            for oc in range(2):
                nc.vector.tensor_tensor(
                    out=mixrep[:, oc],
                    in0=ex[:, oc].unsqueeze(2).to_broadcast([128, G, 12, NB]),
                    in1=rec[:, oc].unsqueeze(2).unsqueeze(3).to_broadcast(
                        [128, G, 12, NB]),
                    op=ALU.mult)
            # mixT[(t,n), oc, o] via PE transpose per (sample, oc);
            # borrows the pool psum bank (free between pool accumulations)
            pt = ps_p.tile([128, 512], F32, tag="pp")
            for jj in range(G):
                mixT = mt.tile([M96, 2, 128], BF16, tag="mixT")
                mixTs[j0 + jj] = mixT
                for oc in range(2):
                    k = 2 * jj + oc
                    ptr = pt[0:M96, 64 * k:64 * (k + 1)].bitcast(BF16)
                    nc.tensor.transpose(
                        ptr, mixrep[:, oc, jj, :, :].rearrange("p a b -> p (a b)"),
                        id_sb)
                    drain(mixT[:, oc, :], ptr)

        def mix_chunk(j, c):
            # c in 0..15: oc = c // 8, hw chunk = c % 8
            oc, k = c // 8, c % 8
            if k == 0:
                ots[(j, oc)] = op.tile([128, HW], BF16, tag="out", name="ot")
            ot = ots[(j, oc)]
            om = ps_m.tile([128, 512], F32, tag="om")
            nc.tensor.matmul(om, mixTs[j][:, oc, :], zts[j][:, 512 * k:512 * (k + 1)],
                             start=True, stop=True)
            drain(ot[:, 512 * k:512 * (k + 1)], om)
            if k == 7:
                nc.scalar.dma_start(out=d_out[j, oc, :, :], in_=ots.pop((j, oc)))
                if oc == 1:
                    zts.pop(j)
                    mixTs.pop(j)

        for j in range(4):
            block_load(j)
        for j in range(BPC):
            if j + 4 < BPC:
                block_load(j + 4)
            pool_list = POOLS_AT.get(j, [])
            mix_list = MIXES_AT.get(j, [])
            # chunk-level interleave: conv chunk + pool chunk + 2 mix chunks
            for k in range(8):
                conv_chunk(j, k)
                for p in pool_list:
                    pool_chunk(p, k)
                if mix_list:
                    mix_chunk(mix_list[0], 2 * k)
                    mix_chunk(mix_list[0], 2 * k + 1)
            conv_finish(j)
            if j in MLP_AT:
                block_mlp(MLP_AT[j])
            # emit remaining mixes (second one in double-mix iters)
            for m in mix_list[1:]:
                for c in range(16):
                    mix_chunk(m, c)

    nc.compile()
    return nc


def _prep_inputs(x, w1, b1, w2, b2, base_filters):
    """Host-side input layout prep. Returns per-core in_maps."""
    B = x.shape[0]
    xs = np.ascontiguousarray(x.reshape(B, C, HW)).astype(ml_dtypes.bfloat16)
    w1t = np.ascontiguousarray(w1.T).astype(np.float32) / float(HW)
    b1c = np.ascontiguousarray(b1.reshape(HID, 1)).astype(np.float32)
    w2p = np.ascontiguousarray(
        w2.reshape(CO, NB, HID).transpose(2, 1, 0)).astype(ml_dtypes.bfloat16)
    # b2r[o_part, oc, smp, n] = b2[(oc*128 + o_part)*8 + n]
    b2r = np.broadcast_to(
        b2.reshape(2, 128, NB).transpose(1, 0, 2)[:, :, None, :],
        (128, 2, G, NB))
    b2r = np.ascontiguousarray(b2r).astype(np.float32)
    filt = base_filters.reshape(NB, CCH, 128, 3, 3)  # [n, cc, cp, dy, dx]
    # ft[c_part, cc, 32*dx + 8*dy + n] = filt[n, cc, c_part, dy, dx]
    ft = np.zeros((128, CCH, M96), dtype=np.float32)
    for dy in range(3):
        for dx in range(3):
            r = TAP_ROW[(dy, dx)]
            ft[:, :, r:r + NB] = filt[:, :, :, dy, dx].transpose(2, 1, 0)
    ft = ft.astype(ml_dtypes.bfloat16)
    ident = np.eye(128, dtype=np.float32).astype(ml_dtypes.bfloat16)
    zeros = np.zeros((128, 66), dtype=ml_dtypes.bfloat16)

    in_maps = []
    for core in range(N_CORES):
        in_maps.append({
            "x": np.ascontiguousarray(xs[core * BPC:(core + 1) * BPC]),
            "w1t": w1t, "b1": b1c, "w2p": w2p, "b2r": b2r,
            "ft": ft, "ident": ident, "zeros": zeros,
        })
    return in_maps


def kernel(x, w1, b1, w2, b2, base_filters):
    global _BUILT
    if _BUILT is None:
        _BUILT = _build()
    nc = _BUILT
    in_maps = _prep_inputs(np.asarray(x, dtype=np.float32),
                           np.asarray(w1, dtype=np.float32),
                           np.asarray(b1, dtype=np.float32),
                           np.asarray(w2, dtype=np.float32),
                           np.asarray(b2, dtype=np.float32),
                           np.asarray(base_filters, dtype=np.float32))
    res = run_bass_kernel_spmd(nc, in_maps, core_ids=list(range(N_CORES)))
    outs = []
    for core in range(N_CORES):
        o = np.asarray(res.results[core]["out"])    # [BPC, 2, 128, HW] bf16
        outs.append(o.reshape(BPC, CO, H, W).astype(np.float32))
    return np.concatenate(outs, axis=0)
